# revision 1
# baseline (speedup 1.0000x reference)
"""DIN-style attention + MLP trunk, Trainium2 Bass kernel, 8-core data parallel.

Shapes (hardcoded): B=32, T=200, TQ=50, E=64, P=128, C=64, U=36.

v2 design notes (cost-model driven):
  * ALL data massaging moves to the host: augR = [itt*D + A_rep ; termq] per
    batch lands via DMA, so mm1 is a single K=65 matmul per tile (the v1
    kernel accumulated a second K=64 matmul against A_rep on-device: 2x the
    PE streaming) and there is no on-device M-build (v1: ~2.8us DVE + 9.8us
    Pool), no transposes, no identity matrix.
  * Everything big ships as bf16: halves DMA bytes, and bf16 moving operands
    stream 1 cycle/row at any width (fp32r needs >=256-wide).  Weights/ACT
    precision is ample for the 2e-2 gate.
  * Broadcast constants (w2 over q-groups) use stride-0 APs instead of
    host-tiled 1800-wide replicas (v1 shipped 1.9MB of pure broadcast).
  * Silu evictions are 900 columns wide (2 psum banks per eviction) to halve
    ACT per-instruction overhead; ACT is the bottleneck engine (~15us).
  * G stays pair-packed (two batches in one 128-row psum tile) so one DVE
    multiply + one grouped reduce cover two batches; the w2 multiplies for
    the middle chunks run on the otherwise-idle Pool engine.
"""

from contextlib import ExitStack

import ml_dtypes
import numpy as np

import concourse.bacc as bacc
import concourse.tile as tile
from concourse import mybir
from concourse.bass_utils import run_bass_kernel_spmd

F32 = mybir.dt.float32
BF16 = mybir.dt.bfloat16
BF16NP = ml_dtypes.bfloat16

B, T, TQ, E = 32, 200, 50, 64
P, C = 128, 64
U = 36
NCORES = 8
BL = B // NCORES  # batches per core
NTQU = TQ * U  # 1800
EPS = 1e-6

TCHUNKS = [(0, 128), (128, 72)]
# mm1 psum tiles are [tsz, 1024] (2 banks); chunks 450 wide at offsets 0/512,
# one 900-wide strided Silu evicts both.  G chunks are 36-multiples aligned to
# the 900-column Silu halves so each G half only waits on that half's Silus.
G_CHUNKS = [(0, 468), (468, 432), (900, 468), (1368, 432)]

# consts pack layout (columns of one [128, 1268] bf16 tensor)
CB0 = 0            # trunk weights: w1f k0 | w1f k1 | w2f k0 | w2f k1 | w3f
W2C = 832          # w2 (36 cols, replicated over partitions)
CH1 = 868          # chunk1 = [up^T[64:128]; cx^T] broadcast over q (200 cols)
CH0 = 1068         # chunk0 = [zeros(interest); up^T[0:64]] broadcast (200)
NCONST = 1268

_CACHE = {}


def _build_program():
    nc = bacc.Bacc(
        "TRN2", target_bir_lowering=False, debug=False, num_devices=NCORES
    )
    d_augL = nc.declare_dram_parameter("augL", [65, BL * T], BF16, isOutput=False)
    d_augR = nc.declare_dram_parameter("augR", [65, BL * NTQU], BF16, isOutput=False)
    d_ubp = nc.declare_dram_parameter(
        "ubp", [128, (BL // 2) * 4 * 128], BF16, isOutput=False
    )
    d_consts = nc.declare_dram_parameter("consts", [128, NCONST], BF16, isOutput=False)
    d_out = nc.declare_dram_parameter("out", [64, BL * TQ], F32, isOutput=True)

    c_dice = float(1.0 / np.sqrt(1.0 + EPS))

    with tile.TileContext(nc) as tc:
        with ExitStack() as ctx:
            singles = ctx.enter_context(tc.tile_pool(name="singles", bufs=1))
            work = ctx.enter_context(tc.tile_pool(name="work", bufs=2))
            ps_z = ctx.enter_context(tc.tile_pool(name="ps_z", bufs=3, space="PSUM"))
            ps_g = ctx.enter_context(tc.tile_pool(name="ps_g", bufs=2, space="PSUM"))

            # All DMAs on one queue so arrival order is exactly priority
            # order: augL + augR b0/b1 gate mm1, consts/ubp are needed from
            # ~8us (G pair 0, trunk), augR b2/b3 from ~6us.
            augR = singles.tile([65, BL * NTQU], BF16)
            nc.sync.dma_start(out=augR[:, 0:NTQU], in_=d_augR[:, 0:NTQU])
            augL = singles.tile([65, BL * T], BF16)
            nc.sync.dma_start(out=augL, in_=d_augL[:])
            nc.sync.dma_start(
                out=augR[:, NTQU:2 * NTQU], in_=d_augR[:, NTQU:2 * NTQU]
            )
            consts = singles.tile([128, NCONST], BF16)
            nc.sync.dma_start(out=consts, in_=d_consts[:])
            ubp = singles.tile([128, (BL // 2) * 4 * 128], BF16)
            nc.sync.dma_start(out=ubp, in_=d_ubp[:])
            nc.sync.dma_start(
                out=augR[:, 2 * NTQU:], in_=d_augR[:, 2 * NTQU:]
            )

            w1f_sb = [consts[:, 0:256], consts[:, 256:512]]
            w2f_sb = [consts[:, 512:640], consts[:, 640:768]]
            w3f_sb = consts[:, 768:832]
            w2_sb = consts[:, W2C:W2C + U]
            chunk1 = consts[:, CH1:CH1 + BL * TQ]
            chunk0 = consts[:, CH0:CH0 + BL * TQ]

            s_tiles = {}

            def mm1_half(ib, h):
                """z = augL_b^T @ augR_b for columns [900h, 900h+900), both
                t-chunks; one 900-wide Silu eviction per t-chunk."""
                if ib not in s_tiles:
                    s_tiles[ib] = [
                        work.tile([128, NTQU], BF16, tag=f"s{ti}_{ib % 2}",
                                  name=f"s{ti}_{ib}")
                        for ti in range(2)
                    ]
                n0 = h * 900
                for ti, (t0, tsz) in enumerate(TCHUNKS):
                    s_t = s_tiles[ib][ti]
                    zp = ps_z.tile([128, 1024], F32, tag="zp")
                    for k in range(2):
                        nc.tensor.matmul(
                            zp[0:tsz, k * 512:k * 512 + 450],
                            augL[:, ib * T + t0:ib * T + t0 + tsz],
                            augR[:, ib * NTQU + n0 + k * 450:
                                 ib * NTQU + n0 + (k + 1) * 450],
                            start=True,
                            stop=True,
                        )
                    nc.scalar.activation(
                        s_t[0:tsz, n0:n0 + 900].rearrange(
                            "p (b c) -> p b c", c=450
                        ),
                        zp[0:tsz].rearrange("p (b c) -> p b c", b=2)[
                            :, :, 0:450
                        ],
                        mybir.ActivationFunctionType.Silu,
                        scale=c_dice,
                    )

            intPs = {}

            def g_half(pb, h):
                """G matmuls + w2 multiply + grouped reduce for the two
                chunks inside Silu half h of pair pb."""
                if pb not in intPs:
                    intPs[pb] = work.tile([128, TQ], F32, tag="intP",
                                          name=f"intP{pb}")
                intP = intPs[pb]
                pbase = pb * 4 * 128
                st = [s_tiles[2 * pb], s_tiles[2 * pb + 1]]
                for ci in (2 * h, 2 * h + 1):
                    n0, nsz = G_CHUNKS[ci]
                    gp = ps_g.tile([128, 512], F32, tag="gp")
                    for k in range(4):
                        tch = k % 2
                        tsz = TCHUNKS[tch][1]
                        nc.tensor.matmul(
                            gp[:, 0:nsz],
                            ubp[0:tsz, pbase + k * 128:pbase + (k + 1) * 128],
                            st[k // 2][tch][0:tsz, n0:n0 + nsz],
                            start=(k == 0),
                            stop=(k == 3),
                        )
                    ng = nsz // U
                    gw = work.tile([128, 504], F32, tag=f"gw{ci % 2}")
                    nc.vector.tensor_tensor(
                        gw[:, 0:nsz].rearrange("e (g u) -> e g u", u=U),
                        gp[:, 0:nsz].rearrange("e (g u) -> e g u", u=U),
                        w2_sb[:, None, :].broadcast_to((128, ng, U)),
                        mybir.AluOpType.mult,
                    )
                    nc.vector.reduce_sum(
                        intP[:, n0 // U:n0 // U + ng],
                        gw[:, 0:nsz].rearrange("e (g u) -> e g u", u=U),
                        axis=mybir.AxisListType.X,
                    )

            def g_fin(pb):
                intP = intPs[pb]
                pair = (2 * pb, 2 * pb + 1)
                nc.vector.tensor_copy(
                    chunk0[0:64, pair[0] * TQ:(pair[0] + 1) * TQ], intP[0:64, :]
                )
                nc.vector.tensor_copy(
                    chunk0[0:64, pair[1] * TQ:(pair[1] + 1) * TQ], intP[64:128, :]
                )

            def trunk_pair(pb):
                n0c = 2 * pb * TQ
                cols = slice(n0c, n0c + 2 * TQ)
                x1 = []
                for mch in range(2):
                    xp = ps_g.tile([128, 512], F32, tag="gp")
                    nc.tensor.matmul(
                        xp[:, 0:2 * TQ],
                        w1f_sb[0][:, mch * 128:(mch + 1) * 128],
                        chunk0[:, cols], start=True, stop=False,
                    )
                    nc.tensor.matmul(
                        xp[:, 0:2 * TQ],
                        w1f_sb[1][:, mch * 128:(mch + 1) * 128],
                        chunk1[:, cols], start=False, stop=True,
                    )
                    x1_t = work.tile([128, 2 * TQ], BF16, tag=f"x1_{mch}")
                    nc.vector.tensor_scalar_max(x1_t, xp[:, 0:2 * TQ], 0.0)
                    x1.append(x1_t)

                xp2 = ps_g.tile([128, 512], F32, tag="gp")
                nc.tensor.matmul(xp2[:, 0:2 * TQ], w2f_sb[0], x1[0],
                                 start=True, stop=False)
                nc.tensor.matmul(xp2[:, 0:2 * TQ], w2f_sb[1], x1[1],
                                 start=False, stop=True)
                x2_t = work.tile([128, 2 * TQ], BF16, tag="x2")
                nc.vector.tensor_scalar_max(x2_t, xp2[:, 0:2 * TQ], 0.0)

                xp3 = ps_g.tile([64, 512], F32, tag="gp")
                nc.tensor.matmul(xp3[:, 0:2 * TQ], w3f_sb, x2_t,
                                 start=True, stop=True)
                out_t = work.tile([64, 2 * TQ], F32, tag="outT")
                nc.vector.tensor_scalar_max(out_t, xp3[:, 0:2 * TQ], 0.0)
                nc.sync.dma_start(out=d_out[:, cols], in_=out_t)

            mm1_half(0, 0)
            mm1_half(1, 0)
            mm1_half(0, 1)
            mm1_half(1, 1)
            g_half(0, 0)
            mm1_half(2, 0)
            g_half(0, 1)
            g_fin(0)
            mm1_half(3, 0)
            mm1_half(2, 1)
            trunk_pair(0)
            mm1_half(3, 1)
            g_half(1, 0)
            g_half(1, 1)
            g_fin(1)
            trunk_pair(1)

    nc.compile()
    return nc


def _prepare_maps(inputs):
    f = lambda k: np.ascontiguousarray(np.asarray(inputs[k], dtype=np.float32))
    W1, W2 = f("W1"), f("W2")
    Wm1, Wm2, Wm3 = f("Wm1"), f("Wm2"), f("Wm3")

    A = W1[0:64] + W1[128:192]     # q rows + (q-k) rows
    Bm = W1[64:128] - W1[128:192]  # k rows - (q-k) rows
    D = W1[192:256]                # (q*k) rows
    c = 1.0 / np.sqrt(1.0 + EPS)   # dice rsqrt(var+eps) with var=1
    cb = 1.0 / np.sqrt(1.0 + EPS)  # BN identity scale

    w1f = cb * Wm1
    w2f = cb * Wm2
    w3f = cb * Wm3
    cB = np.concatenate(
        [w1f[0:128], w1f[128:256], w2f[0:128], w2f[128:256], w3f], axis=1
    )  # (128, 832)
    w2rep = np.tile((W2[:, 0] / c)[None, :], (128, 1))  # (128, 36)

    ub = f("user_behavior")        # (B, T, E)
    it = f("items")                # (B, TQ, E)
    up = f("user_profile")         # (B, P)
    cx = f("context")              # (B, C)

    in_maps = []
    for i in range(NCORES):
        s = slice(i * BL, (i + 1) * BL)
        ub_i, it_i = ub[s], it[s]

        augL = np.empty((65, BL * T), np.float32)
        for b in range(BL):
            augL[0:64, b * T:(b + 1) * T] = ub_i[b].T
            augL[64, b * T:(b + 1) * T] = 1.0

        itt = it_i.transpose(0, 2, 1)  # (BL, E, TQ)
        mprime = (
            itt[:, :, :, None] * D[None, :, None, :]
            + A[None, :, None, :]
        ).reshape(BL, E, NTQU)
        termq = np.einsum("bqe,eu->bqu", it_i, Bm).reshape(BL, NTQU)
        augR = np.empty((65, BL * NTQU), np.float32)
        for b in range(BL):
            augR[0:64, b * NTQU:(b + 1) * NTQU] = mprime[b]
            augR[64, b * NTQU:(b + 1) * NTQU] = termq[b]

        ubp = np.zeros((128, (BL // 2) * 4, 128), np.float32)
        for p in range(BL // 2):
            b0, b1 = 2 * p, 2 * p + 1
            ubp[:, p * 4 + 0, 0:64] = ub_i[b0, 0:128]
            ubp[0:72, p * 4 + 1, 0:64] = ub_i[b0, 128:200]
            ubp[:, p * 4 + 2, 64:128] = ub_i[b1, 0:128]
            ubp[0:72, p * 4 + 3, 64:128] = ub_i[b1, 128:200]

        consts = np.zeros((128, NCONST), np.float32)
        consts[:, 0:832] = cB
        consts[:, W2C:W2C + U] = w2rep
        for b in range(BL):
            cols = slice(CH1 + b * TQ, CH1 + (b + 1) * TQ)
            consts[0:64, cols] = up[s][b, 64:128, None]
            consts[64:128, cols] = cx[s][b, :, None]
            cols = slice(CH0 + b * TQ, CH0 + (b + 1) * TQ)
            consts[64:128, cols] = up[s][b, 0:64, None]

        in_maps.append({
            "augL": np.ascontiguousarray(augL.astype(BF16NP)),
            "augR": np.ascontiguousarray(augR.astype(BF16NP)),
            "ubp": np.ascontiguousarray(
                ubp.reshape(128, (BL // 2) * 4 * 128).astype(BF16NP)
            ),
            "consts": np.ascontiguousarray(consts.astype(BF16NP)),
        })
    return in_maps


def run(inputs, trace=False):
    if "nc" not in _CACHE:
        _CACHE["nc"] = _build_program()
    nc = _CACHE["nc"]
    in_maps = _prepare_maps(inputs)
    res = run_bass_kernel_spmd(nc, in_maps, list(range(NCORES)), trace=trace)
    out = np.empty((B, TQ, 64), dtype=np.float32)
    for i in range(NCORES):
        out[i * BL:(i + 1) * BL] = (
            res.results[i]["out"].T.reshape(BL, TQ, 64)
        )
    return out, res


def kernel(**inputs):
    out, _ = run(inputs, trace=False)
    return out



# revision 4
# speedup vs baseline: 1.1530x; 1.1530x over previous
"""DIN-style attention + MLP trunk, Trainium2 Bass kernel, 8-core data parallel.

Shapes (hardcoded): B=32, T=200, TQ=50, E=64, P=128, C=64, U=36.

v3 design (transposed attention layout, cost-model driven):
  * mm1 computes z^T[(q,u), t] = augR_chunk^T @ augL per batch: stationary =
    augR chunks [65, 120] (qu-rows), moving = augL [65, 200] (t).  PE cols
    per batch: 15 chunks x 200 = 3000 (vs 3600 in the [t,qu] layout), and
    the Silu evictions are 94%-partition-dense: ACT cols 12.6k vs 14.4k.
  * The W2 contraction over u runs on PE as 15 accumulating matmuls per
    (batch, t-chunk) against a host-built block-diagonal selection matrix
    Sel[(qu-row), q] = W2[u]/c (moving, 50 cols) -- this replaces the v2
    DVE tensor_tensor + grouped reduce (9us of DVE) with 6k cheap PE cols.
  * w[t, q] lands directly in [t-part, q-free] orientation (no transpose),
    is evicted to bf16 SBUF, and G = ub^T @ w is 2 matmuls x 50 cols per
    batch (pair-packed into one [128, 50] psum via out-partition offset).
  * termq (+b1) rides as augR row 64 against the augL ones-row, dice's
    rsqrt scale is the Silu `scale`, W2/c is folded into Sel, BN scales
    into the trunk weights.  DMA is compact: no zero-padded ubp, no w2rep.
  * psum: zp pool 2 x [128,1536] (3 banks each), slots hold 6 z-chunks; one
    Silu instruction drains 3 or 6 chunks (600/1200 cols) via a strided AP.
"""

from contextlib import ExitStack

import ml_dtypes
import numpy as np

import concourse.bacc as bacc
import concourse.tile as tile
from concourse import mybir
from concourse.bass_utils import run_bass_kernel_spmd

F32 = mybir.dt.float32
BF16 = mybir.dt.bfloat16
BF16NP = ml_dtypes.bfloat16

B, T, TQ, E = 32, 200, 50, 64
P, C = 128, 64
U = 36
NCORES = 8
BL = B // NCORES   # batches per core
QU = TQ * U        # 1800
CH = 120           # qu-rows per mm1 chunk
NCH = QU // CH     # 15 chunks per batch
EPS = 1e-6

TCH = [(0, 128), (128, 72)]

# z-chunk column offsets inside a [128, 1536] (3-bank) psum tile, ordered so
# any prefix of 3 and the full 6 form rectangular strided APs:
#   first 3: stride 512; 6: [2 x stride 200, 3 x stride 512]
SLOT = [0, 512, 1024, 200, 712, 1224]

# (chunk_start, n_chunks) silu tiles; b0 uses all-3s so ACT starts earlier
B0_TILES = [(0, 3), (3, 3), (6, 3), (9, 3), (12, 3)]
BN_TILES = [(0, 6), (6, 6), (12, 3)]

# consts pack layout (columns of one [128, 1232] bf16 tensor)
CB0 = 0            # trunk weights: w1f k0 | w1f k1 | w2f k0 | w2f k1 | w3f
CH1 = 832          # chunk1 = [up^T[64:128]; cx^T] broadcast over q (200 cols)
CH0 = 1032         # chunk0 = [zeros(interest); up^T[0:64]] broadcast (200)
NCONST = 1232

AUGL = 0           # augLR col layout: augL [65, 800] then augR [65, 7200]
AUGR = BL * T

_CACHE = {}


def _build_program():
    nc = bacc.Bacc(
        "TRN2", target_bir_lowering=False, debug=False, num_devices=NCORES
    )
    d_augLR = nc.declare_dram_parameter(
        "augLR", [65, BL * T + BL * QU], BF16, isOutput=False
    )
    d_sel = nc.declare_dram_parameter("sel", [CH, NCH * TQ], BF16, isOutput=False)
    d_ubt = nc.declare_dram_parameter("ubt", [128, 2 * BL * E], BF16, isOutput=False)
    d_consts = nc.declare_dram_parameter("consts", [128, NCONST], BF16, isOutput=False)
    d_out = nc.declare_dram_parameter("out", [64, BL * TQ], F32, isOutput=True)

    c_dice = float(1.0 / np.sqrt(1.0 + EPS))

    with tile.TileContext(nc) as tc:
        with ExitStack() as ctx:
            singles = ctx.enter_context(tc.tile_pool(name="singles", bufs=1))
            work = ctx.enter_context(tc.tile_pool(name="work", bufs=2))
            ps_z = ctx.enter_context(tc.tile_pool(name="ps_z", bufs=2, space="PSUM"))
            ps_w = ctx.enter_context(tc.tile_pool(name="ps_w", bufs=1, space="PSUM"))
            ps_g = ctx.enter_context(tc.tile_pool(name="ps_g", bufs=1, space="PSUM"))

            augLR = singles.tile([65, BL * T + BL * QU], BF16)
            # first DMA: augL (all batches) + augR b0 chunks 0-2 -> unblocks
            # the first mm1 tile + silu as early as possible
            nc.sync.dma_start(
                out=augLR[:, 0:AUGR + 3 * CH], in_=d_augLR[:, 0:AUGR + 3 * CH]
            )
            nc.sync.dma_start(
                out=augLR[:, AUGR + 3 * CH:AUGR + QU],
                in_=d_augLR[:, AUGR + 3 * CH:AUGR + QU],
            )
            sel = singles.tile([CH, NCH * TQ], BF16)
            nc.sync.dma_start(out=sel, in_=d_sel[:])
            ubt = singles.tile([128, 2 * BL * E], BF16)
            nc.sync.dma_start(out=ubt, in_=d_ubt[:])
            nc.sync.dma_start(
                out=augLR[:, AUGR + QU:AUGR + 2 * QU],
                in_=d_augLR[:, AUGR + QU:AUGR + 2 * QU],
            )
            consts = singles.tile([128, NCONST], BF16)
            nc.sync.dma_start(out=consts, in_=d_consts[:])
            nc.sync.dma_start(
                out=augLR[:, AUGR + 2 * QU:AUGR + 3 * QU],
                in_=d_augLR[:, AUGR + 2 * QU:AUGR + 3 * QU],
            )
            nc.sync.dma_start(
                out=augLR[:, AUGR + 3 * QU:], in_=d_augLR[:, AUGR + 3 * QU:]
            )

            w1f_sb = [consts[:, 0:256], consts[:, 256:512]]
            w2f_sb = [consts[:, 512:640], consts[:, 640:768]]
            w3f_sb = consts[:, 768:832]
            chunk1 = consts[:, CH1:CH1 + BL * TQ]
            chunk0 = consts[:, CH0:CH0 + BL * TQ]

            s_tiles = {}
            w_tiles = {}
            wps = {}

            def mm1_tile(b, t0c, ncb):
                """z^T chunks t0c..t0c+ncb of batch b -> fresh zp tile."""
                zp = ps_z.tile([128, 1536], F32, tag="zp")
                for i in range(ncb):
                    c = t0c + i
                    nc.tensor.matmul(
                        zp[0:CH, SLOT[i]:SLOT[i] + T],
                        augLR[:, AUGR + b * QU + CH * c:AUGR + b * QU + CH * (c + 1)],
                        augLR[:, b * T:(b + 1) * T],
                        start=True,
                        stop=True,
                    )
                return zp

            def silu_tile(b, t0c, ncb, zp):
                """One Silu draining ncb z-chunks (ncb in {3, 6})."""
                if b not in s_tiles:
                    s_tiles[b] = work.tile(
                        [128, NCH * T], BF16, tag=f"s{b % 2}", name=f"s{b}"
                    )
                s_b = s_tiles[b]
                zin = zp[0:CH, :].rearrange("p (a r) -> p a r", r=512)[:, :, 0:400]
                zin = zin.rearrange("p a (two c) -> p two a c", two=2)
                out = s_b[0:CH, T * t0c:T * (t0c + ncb)]
                if ncb == 6:
                    nc.scalar.activation(
                        out.rearrange("p (two a c) -> p two a c", two=2, c=T),
                        zin,
                        mybir.ActivationFunctionType.Silu,
                        scale=c_dice,
                    )
                else:
                    assert ncb == 3
                    nc.scalar.activation(
                        out.rearrange("p (a c) -> p a c", c=T),
                        zin[:, 0],
                        mybir.ActivationFunctionType.Silu,
                        scale=c_dice,
                    )

            def wsel_tile(b, t0c, ncb):
                """Accumulate chunks' W2-contraction into wp (both t-chunks)."""
                if b not in wps:
                    wps[b] = ps_w.tile([128, 128], F32, tag="wp", name=f"wp{b}")
                wp = wps[b]
                s_b = s_tiles[b]
                # Single start=True on the very first matmul: its lazy
                # pending-zero covers the whole bank (ZERO_REGION=2KB), so
                # tch1's first start=False write lands on pending-zero bytes
                # and overwrites.  Per-region start=True would instead wipe
                # tch0's partial accumulation (bank-granular zeroing).
                for tch, (tc0, tlen) in enumerate(TCH):
                    for i in range(ncb):
                        c = t0c + i
                        nc.tensor.matmul(
                            wp[0:tlen, 64 * tch:64 * tch + TQ],
                            s_b[0:CH, T * c + tc0:T * c + tc0 + tlen],
                            sel[:, TQ * c:TQ * (c + 1)],
                            start=(tch == 0 and c == 0),
                            stop=(tch == 1 and c == NCH - 1),
                            skip_group_check=True,
                        )

            def wevict(b):
                w_tiles[b] = work.tile(
                    [128, 128], BF16, tag=f"w{b % 2}", name=f"w{b}"
                )
                wp = wps[b]
                nc.vector.tensor_copy(w_tiles[b][:, 0:TQ], wp[:, 0:TQ])
                nc.vector.tensor_copy(
                    w_tiles[b][0:72, 64:64 + TQ], wp[0:72, 64:64 + TQ]
                )

            def g_pair(pb):
                """interest for pair (2pb, 2pb+1), pair-packed [128, 50]."""
                gp = ps_g.tile([128, 512], F32, tag="gx")
                for half in range(2):
                    b = 2 * pb + half
                    for tch, (tc0, tlen) in enumerate(TCH):
                        nc.tensor.matmul(
                            gp[64 * half:64 * half + 64, 0:TQ],
                            ubt[0:tlen, tch * BL * E + b * E:tch * BL * E + (b + 1) * E],
                            w_tiles[b][0:tlen, 64 * tch:64 * tch + TQ],
                            start=(tch == 0),
                            stop=(tch == 1),
                        )
                pair = (2 * pb, 2 * pb + 1)
                nc.vector.tensor_copy(
                    chunk0[0:64, pair[0] * TQ:(pair[0] + 1) * TQ], gp[0:64, 0:TQ]
                )
                nc.vector.tensor_copy(
                    chunk0[0:64, pair[1] * TQ:(pair[1] + 1) * TQ], gp[64:128, 0:TQ]
                )

            def trunk_pair(pb):
                n0c = 2 * pb * TQ
                cols = slice(n0c, n0c + 2 * TQ)
                x1 = []
                for mch in range(2):
                    xp = ps_g.tile([128, 512], F32, tag="gx")
                    nc.tensor.matmul(
                        xp[:, 0:2 * TQ],
                        w1f_sb[0][:, mch * 128:(mch + 1) * 128],
                        chunk0[:, cols], start=True, stop=False,
                    )
                    nc.tensor.matmul(
                        xp[:, 0:2 * TQ],
                        w1f_sb[1][:, mch * 128:(mch + 1) * 128],
                        chunk1[:, cols], start=False, stop=True,
                    )
                    x1_t = work.tile([128, 2 * TQ], BF16, tag=f"x1_{mch}")
                    nc.vector.tensor_scalar_max(x1_t, xp[:, 0:2 * TQ], 0.0)
                    x1.append(x1_t)

                xp2 = ps_g.tile([128, 512], F32, tag="gx")
                nc.tensor.matmul(xp2[:, 0:2 * TQ], w2f_sb[0], x1[0],
                                 start=True, stop=False)
                nc.tensor.matmul(xp2[:, 0:2 * TQ], w2f_sb[1], x1[1],
                                 start=False, stop=True)
                x2_t = work.tile([128, 2 * TQ], BF16, tag="x2")
                nc.vector.tensor_scalar_max(x2_t, xp2[:, 0:2 * TQ], 0.0)

                xp3 = ps_g.tile([64, 512], F32, tag="gx")
                nc.tensor.matmul(xp3[:, 0:2 * TQ], w3f_sb, x2_t,
                                 start=True, stop=True)
                out_t = work.tile([64, 2 * TQ], F32, tag="outT")
                nc.vector.tensor_scalar_max(out_t, xp3[:, 0:2 * TQ], 0.0)
                nc.sync.dma_start(out=d_out[:, cols], in_=out_t)

            def batch(b, tiles, pre_hooks=()):
                hooks = dict(pre_hooks)
                done = 0
                pend = []  # (t0c, ncb) silu'd, wsel pending
                for ti, (t0c, ncb) in enumerate(tiles):
                    zp = mm1_tile(b, t0c, ncb)
                    silu_tile(b, t0c, ncb, zp)
                    if ti in hooks:
                        hooks[ti]()
                    # run wsel one tile behind so PE isn't blocked on ACT
                    if pend and ti >= 1:
                        wsel_tile(b, *pend.pop(0))
                    pend.append((t0c, ncb))
                    done += 1
                for args in pend:
                    wsel_tile(b, *args)
                wevict(b)

            batch(0, B0_TILES)
            batch(1, BN_TILES)
            g_pair(0)
            batch(2, BN_TILES, pre_hooks={1: lambda: trunk_pair(0)})
            batch(3, BN_TILES)
            g_pair(1)
            trunk_pair(1)

    nc.compile()
    return nc


def _prepare_maps(inputs):
    f = lambda k: np.ascontiguousarray(np.asarray(inputs[k], dtype=np.float32))
    W1, W2 = f("W1"), f("W2")
    b1 = f("b1")
    Wm1, Wm2, Wm3 = f("Wm1"), f("Wm2"), f("Wm3")

    A = W1[0:64] + W1[128:192]     # q rows + (q-k) rows
    Bm = W1[64:128] - W1[128:192]  # k rows - (q-k) rows
    D = W1[192:256]                # (q*k) rows
    c = 1.0 / np.sqrt(1.0 + EPS)   # dice rsqrt(var+eps) with var=1
    cb = 1.0 / np.sqrt(1.0 + EPS)  # BN identity scale

    w1f = cb * Wm1
    w2f = cb * Wm2
    w3f = cb * Wm3
    cB = np.concatenate(
        [w1f[0:128], w1f[128:256], w2f[0:128], w2f[128:256], w3f], axis=1
    )  # (128, 832)

    # Sel[(qu-row within chunk), 50c + q] = W2[u]/c * delta(q == r//36)
    M = np.zeros((QU, TQ), np.float32)
    r = np.arange(QU)
    M[r, r // U] = W2[r % U, 0] / c
    selm = M.reshape(NCH, CH, TQ).transpose(1, 0, 2).reshape(CH, NCH * TQ)

    ub = f("user_behavior")        # (B, T, E)
    it = f("items")                # (B, TQ, E)
    up = f("user_profile")         # (B, P)
    cx = f("context")              # (B, C)

    in_maps = []
    for i in range(NCORES):
        s = slice(i * BL, (i + 1) * BL)
        ub_i, it_i = ub[s], it[s]

        augLR = np.empty((65, BL * T + BL * QU), np.float32)
        for b in range(BL):
            augLR[0:64, b * T:(b + 1) * T] = ub_i[b].T
            augLR[64, b * T:(b + 1) * T] = 1.0

        itt = it_i.transpose(0, 2, 1)  # (BL, E, TQ)
        mprime = (
            itt[:, :, :, None] * D[None, :, None, :]
            + A[None, :, None, :]
        ).reshape(BL, E, QU)
        termq = (
            np.einsum("bqe,eu->bqu", it_i, Bm) + b1[None, None, :]
        ).reshape(BL, QU)
        for b in range(BL):
            cols = slice(AUGR + b * QU, AUGR + (b + 1) * QU)
            augLR[0:64, cols] = mprime[b]
            augLR[64, cols] = termq[b]

        ubt = np.zeros((128, 2 * BL * E), np.float32)
        for b in range(BL):
            ubt[0:128, b * E:(b + 1) * E] = ub_i[b, 0:128]
            ubt[0:72, BL * E + b * E:BL * E + (b + 1) * E] = ub_i[b, 128:200]

        consts = np.zeros((128, NCONST), np.float32)
        consts[:, 0:832] = cB
        for b in range(BL):
            cols = slice(CH1 + b * TQ, CH1 + (b + 1) * TQ)
            consts[0:64, cols] = up[s][b, 64:128, None]
            consts[64:128, cols] = cx[s][b, :, None]
            cols = slice(CH0 + b * TQ, CH0 + (b + 1) * TQ)
            consts[64:128, cols] = up[s][b, 0:64, None]

        in_maps.append({
            "augLR": np.ascontiguousarray(augLR.astype(BF16NP)),
            "sel": np.ascontiguousarray(selm.astype(BF16NP)),
            "ubt": np.ascontiguousarray(ubt.astype(BF16NP)),
            "consts": np.ascontiguousarray(consts.astype(BF16NP)),
        })
    return in_maps


def run(inputs, trace=False):
    if "nc" not in _CACHE:
        _CACHE["nc"] = _build_program()
    nc = _CACHE["nc"]
    in_maps = _prepare_maps(inputs)
    res = run_bass_kernel_spmd(nc, in_maps, list(range(NCORES)), trace=trace)
    out = np.empty((B, TQ, 64), dtype=np.float32)
    for i in range(NCORES):
        out[i * BL:(i + 1) * BL] = (
            res.results[i]["out"].T.reshape(BL, TQ, 64)
        )
    return out, res


def kernel(**inputs):
    out, _ = run(inputs, trace=False)
    return out


# revision 5
# speedup vs baseline: 1.1902x; 1.0322x over previous
"""DIN-style attention + MLP trunk, Trainium2 Bass kernel, 8-core data parallel.

Shapes (hardcoded): B=32, T=200, TQ=50, E=64, P=128, C=64, U=36.

v3 design (transposed attention layout, cost-model driven):
  * mm1 computes z^T[(q,u), t] = augR_chunk^T @ augL per batch: stationary =
    augR chunks [65, 120] (qu-rows), moving = augL [65, 200] (t).  PE cols
    per batch: 15 chunks x 200 = 3000 (vs 3600 in the [t,qu] layout), and
    the Silu evictions are 94%-partition-dense: ACT cols 12.6k vs 14.4k.
  * The W2 contraction over u runs on PE as 15 accumulating matmuls per
    (batch, t-chunk) against a host-built block-diagonal selection matrix
    Sel[(qu-row), q] = W2[u]/c (moving, 50 cols) -- this replaces the v2
    DVE tensor_tensor + grouped reduce (9us of DVE) with 6k cheap PE cols.
  * w[t, q] lands directly in [t-part, q-free] orientation (no transpose),
    is evicted to bf16 SBUF, and G = ub^T @ w is 2 matmuls x 50 cols per
    batch (pair-packed into one [128, 50] psum via out-partition offset).
  * termq (+b1) rides as augR row 64 against the augL ones-row, dice's
    rsqrt scale is the Silu `scale`, W2/c is folded into Sel, BN scales
    into the trunk weights.  DMA is compact: no zero-padded ubp, no w2rep.
  * psum: zp pool 2 x [128,1536] (3 banks each), slots hold 6 z-chunks; one
    Silu instruction drains 3 or 6 chunks (600/1200 cols) via a strided AP.
"""

from contextlib import ExitStack

import ml_dtypes
import numpy as np

import concourse.bacc as bacc
import concourse.tile as tile
from concourse import mybir
from concourse.bass_utils import run_bass_kernel_spmd

F32 = mybir.dt.float32
BF16 = mybir.dt.bfloat16
BF16NP = ml_dtypes.bfloat16

B, T, TQ, E = 32, 200, 50, 64
P, C = 128, 64
U = 36
NCORES = 8
BL = B // NCORES   # batches per core
QU = TQ * U        # 1800
CH = 120           # qu-rows per mm1 chunk
NCH = QU // CH     # 15 chunks per batch
EPS = 1e-6

TCH = [(0, 128), (128, 72)]

# z-chunk column offsets inside a [128, 1536] (3-bank) psum tile, ordered so
# any prefix of 3 and the full 6 form rectangular strided APs:
#   first 3: stride 512; 6: [2 x stride 200, 3 x stride 512]
SLOT = [0, 512, 1024, 200, 712, 1224]

# (chunk_start, n_chunks) silu tiles; b0 uses all-3s so ACT starts earlier
B0_TILES = [(0, 3), (3, 3), (6, 3), (9, 3), (12, 3)]
BN_TILES = [(0, 6), (6, 6), (12, 3)]

# consts pack layout (columns of one [128, 1232] bf16 tensor)
CB0 = 0            # trunk weights: w1f k0 | w1f k1 | w2f k0 | w2f k1 | w3f
CH1 = 832          # chunk1 = [up^T[64:128]; cx^T] broadcast over q (200 cols)
CH0 = 1032         # chunk0 = [zeros(interest); up^T[0:64]] broadcast (200)
NCONST = 1232

AUGL = 0           # augLR col layout: augL [65, 800] then augR [65, 7200]
AUGR = BL * T

_CACHE = {}


def _build_program():
    nc = bacc.Bacc(
        "TRN2", target_bir_lowering=False, debug=False, num_devices=NCORES
    )
    d_augLR = nc.declare_dram_parameter(
        "augLR", [65, BL * T + BL * QU], BF16, isOutput=False
    )
    d_sel = nc.declare_dram_parameter("sel", [CH, NCH * TQ], BF16, isOutput=False)
    d_ubt = nc.declare_dram_parameter("ubt", [128, 2 * BL * E], BF16, isOutput=False)
    d_consts = nc.declare_dram_parameter("consts", [128, NCONST], BF16, isOutput=False)
    d_out = nc.declare_dram_parameter("out", [64, BL * TQ], F32, isOutput=True)

    c_dice = float(1.0 / np.sqrt(1.0 + EPS))

    with tile.TileContext(nc) as tc:
        with ExitStack() as ctx:
            singles = ctx.enter_context(tc.tile_pool(name="singles", bufs=1))
            work = ctx.enter_context(tc.tile_pool(name="work", bufs=2))
            ps_z = ctx.enter_context(tc.tile_pool(name="ps_z", bufs=2, space="PSUM"))
            ps_w = ctx.enter_context(tc.tile_pool(name="ps_w", bufs=1, space="PSUM"))
            ps_g = ctx.enter_context(tc.tile_pool(name="ps_g", bufs=1, space="PSUM"))

            augLR = singles.tile([65, BL * T + BL * QU], BF16)
            # first DMA: augL (all batches) + augR b0 chunks 0-2 -> unblocks
            # the first mm1 tile + silu as early as possible
            nc.sync.dma_start(
                out=augLR[:, 0:AUGR + 3 * CH], in_=d_augLR[:, 0:AUGR + 3 * CH]
            )
            nc.sync.dma_start(
                out=augLR[:, AUGR + 3 * CH:AUGR + QU],
                in_=d_augLR[:, AUGR + 3 * CH:AUGR + QU],
            )
            sel = singles.tile([CH, NCH * TQ], BF16)
            nc.sync.dma_start(out=sel, in_=d_sel[:])
            ubt = singles.tile([128, 2 * BL * E], BF16)
            nc.sync.dma_start(out=ubt, in_=d_ubt[:])
            nc.sync.dma_start(
                out=augLR[:, AUGR + QU:AUGR + 2 * QU],
                in_=d_augLR[:, AUGR + QU:AUGR + 2 * QU],
            )
            consts = singles.tile([128, NCONST], BF16)
            nc.sync.dma_start(out=consts, in_=d_consts[:])
            nc.sync.dma_start(
                out=augLR[:, AUGR + 2 * QU:AUGR + 3 * QU],
                in_=d_augLR[:, AUGR + 2 * QU:AUGR + 3 * QU],
            )
            nc.sync.dma_start(
                out=augLR[:, AUGR + 3 * QU:], in_=d_augLR[:, AUGR + 3 * QU:]
            )

            w1f_sb = [consts[:, 0:256], consts[:, 256:512]]
            w2f_sb = [consts[:, 512:640], consts[:, 640:768]]
            w3f_sb = consts[:, 768:832]
            chunk1 = consts[:, CH1:CH1 + BL * TQ]
            chunk0 = consts[:, CH0:CH0 + BL * TQ]

            s_tiles = {}
            w_tiles = {}
            wps = {}

            def mm1_tile(b, t0c, ncb):
                """z^T chunks t0c..t0c+ncb of batch b -> fresh zp tile."""
                zp = ps_z.tile([128, 1536], F32, tag="zp")
                for i in range(ncb):
                    c = t0c + i
                    nc.tensor.matmul(
                        zp[0:CH, SLOT[i]:SLOT[i] + T],
                        augLR[:, AUGR + b * QU + CH * c:AUGR + b * QU + CH * (c + 1)],
                        augLR[:, b * T:(b + 1) * T],
                        start=True,
                        stop=True,
                    )
                return zp

            def silu_tile(b, t0c, ncb, zp):
                """One Silu draining ncb z-chunks (ncb in {3, 6})."""
                if b not in s_tiles:
                    s_tiles[b] = work.tile(
                        [128, NCH * T], BF16, tag=f"s{b % 2}", name=f"s{b}"
                    )
                s_b = s_tiles[b]
                zin = zp[0:CH, :].rearrange("p (a r) -> p a r", r=512)[:, :, 0:400]
                zin = zin.rearrange("p a (two c) -> p two a c", two=2)
                out = s_b[0:CH, T * t0c:T * (t0c + ncb)]
                if ncb == 6:
                    nc.scalar.activation(
                        out.rearrange("p (two a c) -> p two a c", two=2, c=T),
                        zin,
                        mybir.ActivationFunctionType.Silu,
                        scale=c_dice,
                    )
                else:
                    assert ncb == 3
                    nc.scalar.activation(
                        out.rearrange("p (a c) -> p a c", c=T),
                        zin[:, 0],
                        mybir.ActivationFunctionType.Silu,
                        scale=c_dice,
                    )

            def wsel_tile(b, t0c, ncb):
                """Accumulate chunks' W2-contraction into wp (both t-chunks)."""
                if b not in wps:
                    wps[b] = ps_w.tile([128, 128], F32, tag="wp", name=f"wp{b}")
                wp = wps[b]
                s_b = s_tiles[b]
                # Single start=True on the very first matmul: its lazy
                # pending-zero covers the whole bank (ZERO_REGION=2KB), so
                # tch1's first start=False write lands on pending-zero bytes
                # and overwrites.  Per-region start=True would instead wipe
                # tch0's partial accumulation (bank-granular zeroing).
                for tch, (tc0, tlen) in enumerate(TCH):
                    for i in range(ncb):
                        c = t0c + i
                        nc.tensor.matmul(
                            wp[0:tlen, 64 * tch:64 * tch + TQ],
                            s_b[0:CH, T * c + tc0:T * c + tc0 + tlen],
                            sel[:, TQ * c:TQ * (c + 1)],
                            start=(tch == 0 and c == 0),
                            stop=(tch == 1 and c == NCH - 1),
                            skip_group_check=True,
                        )

            def wevict(b):
                w_tiles[b] = work.tile(
                    [128, 128], BF16, tag=f"w{b % 2}", name=f"w{b}"
                )
                wp = wps[b]
                nc.vector.tensor_copy(w_tiles[b][:, 0:TQ], wp[:, 0:TQ])
                nc.vector.tensor_copy(
                    w_tiles[b][0:72, 64:64 + TQ], wp[0:72, 64:64 + TQ]
                )

            def g_pair(pb):
                """interest for pair (2pb, 2pb+1), pair-packed [128, 50]."""
                gp = ps_g.tile([128, 512], F32, tag="gx")
                for half in range(2):
                    b = 2 * pb + half
                    for tch, (tc0, tlen) in enumerate(TCH):
                        nc.tensor.matmul(
                            gp[64 * half:64 * half + 64, 0:TQ],
                            ubt[0:tlen, tch * BL * E + b * E:tch * BL * E + (b + 1) * E],
                            w_tiles[b][0:tlen, 64 * tch:64 * tch + TQ],
                            start=(tch == 0),
                            stop=(tch == 1),
                        )
                pair = (2 * pb, 2 * pb + 1)
                nc.vector.tensor_copy(
                    chunk0[0:64, pair[0] * TQ:(pair[0] + 1) * TQ], gp[0:64, 0:TQ]
                )
                nc.vector.tensor_copy(
                    chunk0[0:64, pair[1] * TQ:(pair[1] + 1) * TQ], gp[64:128, 0:TQ]
                )

            def relu_evict(out_t, xp_ap, on_act):
                if on_act:
                    nc.scalar.activation(
                        out_t, xp_ap, mybir.ActivationFunctionType.Relu
                    )
                else:
                    nc.vector.tensor_scalar_max(out_t, xp_ap, 0.0)

            def trunk_pair(pb, tail=False):
                """tail=True (last pair): second psum slot (freed wp bank) +
                ACT for half the evictions, parallelizing the stage chain."""
                n0c = 2 * pb * TQ
                cols = slice(n0c, n0c + 2 * TQ)
                x1 = []
                for mch in range(2):
                    if tail and mch == 1:
                        xp = ps_w.tile([128, 128], F32, tag="wp")
                    else:
                        xp = ps_g.tile([128, 512], F32, tag="gx")
                    nc.tensor.matmul(
                        xp[:, 0:2 * TQ],
                        w1f_sb[0][:, mch * 128:(mch + 1) * 128],
                        chunk0[:, cols], start=True, stop=False,
                    )
                    nc.tensor.matmul(
                        xp[:, 0:2 * TQ],
                        w1f_sb[1][:, mch * 128:(mch + 1) * 128],
                        chunk1[:, cols], start=False, stop=True,
                    )
                    x1_t = work.tile([128, 2 * TQ], BF16, tag=f"x1_{mch}")
                    relu_evict(x1_t, xp[:, 0:2 * TQ], tail and mch == 1)
                    x1.append(x1_t)

                xp2 = ps_g.tile([128, 512], F32, tag="gx")
                nc.tensor.matmul(xp2[:, 0:2 * TQ], w2f_sb[0], x1[0],
                                 start=True, stop=False)
                nc.tensor.matmul(xp2[:, 0:2 * TQ], w2f_sb[1], x1[1],
                                 start=False, stop=True)
                x2_t = work.tile([128, 2 * TQ], BF16, tag="x2")
                relu_evict(x2_t, xp2[:, 0:2 * TQ], False)

                if tail:
                    xp3 = ps_w.tile([64, 128], F32, tag="wp")
                else:
                    xp3 = ps_g.tile([64, 512], F32, tag="gx")
                nc.tensor.matmul(xp3[:, 0:2 * TQ], w3f_sb, x2_t,
                                 start=True, stop=True)
                out_t = work.tile([64, 2 * TQ], F32, tag="outT")
                relu_evict(out_t, xp3[:, 0:2 * TQ], tail)
                nc.sync.dma_start(out=d_out[:, cols], in_=out_t)

            def tile_step(b, t0c, ncb):
                zp = mm1_tile(b, t0c, ncb)
                silu_tile(b, t0c, ncb, zp)

            # Software-pipelined schedule.  Cross-batch: the next batch's
            # first mm1+silu tile is emitted BEFORE the previous batch's
            # trailing wsel matmuls so ACT never starves at batch borders.
            tile_step(0, 0, 3)
            tile_step(0, 3, 3)
            tile_step(0, 6, 3)
            wsel_tile(0, 0, 3)
            tile_step(0, 9, 3)
            wsel_tile(0, 3, 3)
            tile_step(0, 12, 3)
            wsel_tile(0, 6, 3)
            tile_step(1, 0, 6)
            wsel_tile(0, 9, 3)
            wsel_tile(0, 12, 3)
            wevict(0)
            tile_step(1, 6, 6)
            wsel_tile(1, 0, 6)
            tile_step(1, 12, 3)
            wsel_tile(1, 6, 6)
            tile_step(2, 0, 6)
            wsel_tile(1, 12, 3)
            wevict(1)
            g_pair(0)
            tile_step(2, 6, 6)
            wsel_tile(2, 0, 6)
            trunk_pair(0)
            tile_step(2, 12, 3)
            wsel_tile(2, 6, 6)
            tile_step(3, 0, 6)
            wsel_tile(2, 12, 3)
            wevict(2)
            tile_step(3, 6, 6)
            wsel_tile(3, 0, 6)
            tile_step(3, 12, 3)
            wsel_tile(3, 6, 6)
            wsel_tile(3, 12, 3)
            wevict(3)
            g_pair(1)
            trunk_pair(1, tail=True)

    nc.compile()
    return nc


def _prepare_maps(inputs):
    f = lambda k: np.ascontiguousarray(np.asarray(inputs[k], dtype=np.float32))
    W1, W2 = f("W1"), f("W2")
    b1 = f("b1")
    Wm1, Wm2, Wm3 = f("Wm1"), f("Wm2"), f("Wm3")

    A = W1[0:64] + W1[128:192]     # q rows + (q-k) rows
    Bm = W1[64:128] - W1[128:192]  # k rows - (q-k) rows
    D = W1[192:256]                # (q*k) rows
    c = 1.0 / np.sqrt(1.0 + EPS)   # dice rsqrt(var+eps) with var=1
    cb = 1.0 / np.sqrt(1.0 + EPS)  # BN identity scale

    w1f = cb * Wm1
    w2f = cb * Wm2
    w3f = cb * Wm3
    cB = np.concatenate(
        [w1f[0:128], w1f[128:256], w2f[0:128], w2f[128:256], w3f], axis=1
    )  # (128, 832)

    # Sel[(qu-row within chunk), 50c + q] = W2[u]/c * delta(q == r//36)
    M = np.zeros((QU, TQ), np.float32)
    r = np.arange(QU)
    M[r, r // U] = W2[r % U, 0] / c
    selm = M.reshape(NCH, CH, TQ).transpose(1, 0, 2).reshape(CH, NCH * TQ)

    ub = f("user_behavior")        # (B, T, E)
    it = f("items")                # (B, TQ, E)
    up = f("user_profile")         # (B, P)
    cx = f("context")              # (B, C)

    in_maps = []
    for i in range(NCORES):
        s = slice(i * BL, (i + 1) * BL)
        ub_i, it_i = ub[s], it[s]

        augLR = np.empty((65, BL * T + BL * QU), np.float32)
        for b in range(BL):
            augLR[0:64, b * T:(b + 1) * T] = ub_i[b].T
            augLR[64, b * T:(b + 1) * T] = 1.0

        itt = it_i.transpose(0, 2, 1)  # (BL, E, TQ)
        mprime = (
            itt[:, :, :, None] * D[None, :, None, :]
            + A[None, :, None, :]
        ).reshape(BL, E, QU)
        termq = (
            np.einsum("bqe,eu->bqu", it_i, Bm) + b1[None, None, :]
        ).reshape(BL, QU)
        for b in range(BL):
            cols = slice(AUGR + b * QU, AUGR + (b + 1) * QU)
            augLR[0:64, cols] = mprime[b]
            augLR[64, cols] = termq[b]

        ubt = np.zeros((128, 2 * BL * E), np.float32)
        for b in range(BL):
            ubt[0:128, b * E:(b + 1) * E] = ub_i[b, 0:128]
            ubt[0:72, BL * E + b * E:BL * E + (b + 1) * E] = ub_i[b, 128:200]

        consts = np.zeros((128, NCONST), np.float32)
        consts[:, 0:832] = cB
        for b in range(BL):
            cols = slice(CH1 + b * TQ, CH1 + (b + 1) * TQ)
            consts[0:64, cols] = up[s][b, 64:128, None]
            consts[64:128, cols] = cx[s][b, :, None]
            cols = slice(CH0 + b * TQ, CH0 + (b + 1) * TQ)
            consts[64:128, cols] = up[s][b, 0:64, None]

        in_maps.append({
            "augLR": np.ascontiguousarray(augLR.astype(BF16NP)),
            "sel": np.ascontiguousarray(selm.astype(BF16NP)),
            "ubt": np.ascontiguousarray(ubt.astype(BF16NP)),
            "consts": np.ascontiguousarray(consts.astype(BF16NP)),
        })
    return in_maps


def run(inputs, trace=False):
    if "nc" not in _CACHE:
        _CACHE["nc"] = _build_program()
    nc = _CACHE["nc"]
    in_maps = _prepare_maps(inputs)
    res = run_bass_kernel_spmd(nc, in_maps, list(range(NCORES)), trace=trace)
    out = np.empty((B, TQ, 64), dtype=np.float32)
    for i in range(NCORES):
        out[i * BL:(i + 1) * BL] = (
            res.results[i]["out"].T.reshape(BL, TQ, 64)
        )
    return out, res


def kernel(**inputs):
    out, _ = run(inputs, trace=False)
    return out


# revision 8
# speedup vs baseline: 1.2102x; 1.0169x over previous
"""DIN-style attention + MLP trunk, Trainium2 Bass kernel, 8-core data parallel.

Shapes (hardcoded): B=32, T=200, TQ=50, E=64, P=128, C=64, U=36.

v3 design (transposed attention layout, cost-model driven):
  * mm1 computes z^T[(q,u), t] = augR_chunk^T @ augL per batch: stationary =
    augR chunks [65, 120] (qu-rows), moving = augL [65, 200] (t).  PE cols
    per batch: 15 chunks x 200 = 3000 (vs 3600 in the [t,qu] layout), and
    the Silu evictions are 94%-partition-dense: ACT cols 12.6k vs 14.4k.
  * The W2 contraction over u runs on PE as 15 accumulating matmuls per
    (batch, t-chunk) against a host-built block-diagonal selection matrix
    Sel[(qu-row), q] = W2[u]/c (moving, 50 cols) -- this replaces the v2
    DVE tensor_tensor + grouped reduce (9us of DVE) with 6k cheap PE cols.
  * w[t, q] lands directly in [t-part, q-free] orientation (no transpose),
    is evicted to bf16 SBUF, and G = ub^T @ w is 2 matmuls x 50 cols per
    batch (pair-packed into one [128, 50] psum via out-partition offset).
  * termq (+b1) rides as augR row 64 against the augL ones-row, dice's
    rsqrt scale is the Silu `scale`, W2/c is folded into Sel, BN scales
    into the trunk weights.  DMA is compact: no zero-padded ubp, no w2rep.
  * psum: zp pool 2 x [128,1536] (3 banks each), slots hold 6 z-chunks; one
    Silu instruction drains 3 or 6 chunks (600/1200 cols) via a strided AP.
"""

from contextlib import ExitStack

import ml_dtypes
import numpy as np

import concourse.bacc as bacc
import concourse.tile as tile
from concourse import mybir
from concourse.bass_utils import run_bass_kernel_spmd

F32 = mybir.dt.float32
BF16 = mybir.dt.bfloat16
BF16NP = ml_dtypes.bfloat16

B, T, TQ, E = 32, 200, 50, 64
P, C = 128, 64
U = 36
NCORES = 8
BL = B // NCORES   # batches per core
QU = TQ * U        # 1800
CH = 120           # qu-rows per mm1 chunk
NCH = QU // CH     # 15 chunks per batch
EPS = 1e-6

TCH = [(0, 128), (128, 72)]

# z-chunk column offsets inside a [128, 1536] (3-bank) psum tile, ordered so
# any prefix of 3 and the full 6 form rectangular strided APs:
#   first 3: stride 512; 6: [2 x stride 200, 3 x stride 512]
SLOT = [0, 512, 1024, 200, 712, 1224]

# (chunk_start, n_chunks) silu tiles; b0 uses all-3s so ACT starts earlier
B0_TILES = [(0, 3), (3, 3), (6, 3), (9, 3), (12, 3)]
BN_TILES = [(0, 6), (6, 6), (12, 3)]

# consts pack layout (columns of one [128, 1232] bf16 tensor)
CB0 = 0            # trunk weights: w1f k0 | w1f k1 | w2f k0 | w2f k1 | w3f
CH1 = 832          # chunk1 = [up^T[64:128]; cx^T] broadcast over q (200 cols)
CH0 = 1032         # chunk0 = [zeros(interest); up^T[0:64]] broadcast (200)
NCONST = 1232

AUGL = 0           # augLR col layout: augL [65, 800] then augR [65, 7200]
AUGR = BL * T

_CACHE = {}


def _build_program():
    nc = bacc.Bacc(
        "TRN2", target_bir_lowering=False, debug=False, num_devices=NCORES
    )
    d_augLR = nc.declare_dram_parameter(
        "augLR", [65, BL * T + BL * QU], BF16, isOutput=False
    )
    d_sel = nc.declare_dram_parameter("sel", [CH, NCH * TQ], BF16, isOutput=False)
    d_ubt = nc.declare_dram_parameter("ubt", [128, 2 * BL * E], BF16, isOutput=False)
    d_consts = nc.declare_dram_parameter("consts", [128, NCONST], BF16, isOutput=False)
    d_out = nc.declare_dram_parameter("out", [64, BL * TQ], F32, isOutput=True)

    c_dice = float(1.0 / np.sqrt(1.0 + EPS))

    with tile.TileContext(nc) as tc:
        with ExitStack() as ctx:
            singles = ctx.enter_context(tc.tile_pool(name="singles", bufs=1))
            work = ctx.enter_context(tc.tile_pool(name="work", bufs=2))
            ps_z = ctx.enter_context(tc.tile_pool(name="ps_z", bufs=2, space="PSUM"))
            ps_w = ctx.enter_context(tc.tile_pool(name="ps_w", bufs=1, space="PSUM"))
            ps_g = ctx.enter_context(tc.tile_pool(name="ps_g", bufs=1, space="PSUM"))

            augLR = singles.tile([65, BL * T + BL * QU], BF16)
            # first DMA: augL (all batches) + augR b0 chunks 0-2 -> unblocks
            # the first mm1 tile + silu as early as possible
            nc.sync.dma_start(
                out=augLR[:, 0:AUGR + 3 * CH], in_=d_augLR[:, 0:AUGR + 3 * CH]
            )
            nc.sync.dma_start(
                out=augLR[:, AUGR + 3 * CH:AUGR + QU],
                in_=d_augLR[:, AUGR + 3 * CH:AUGR + QU],
            )
            sel = singles.tile([CH, NCH * TQ], BF16)
            nc.sync.dma_start(out=sel, in_=d_sel[:])
            ubt = singles.tile([128, 2 * BL * E], BF16)
            nc.sync.dma_start(out=ubt, in_=d_ubt[:])
            nc.sync.dma_start(
                out=augLR[:, AUGR + QU:AUGR + 2 * QU],
                in_=d_augLR[:, AUGR + QU:AUGR + 2 * QU],
            )
            consts = singles.tile([128, NCONST], BF16)
            nc.sync.dma_start(out=consts, in_=d_consts[:])
            nc.sync.dma_start(
                out=augLR[:, AUGR + 2 * QU:AUGR + 3 * QU],
                in_=d_augLR[:, AUGR + 2 * QU:AUGR + 3 * QU],
            )
            nc.sync.dma_start(
                out=augLR[:, AUGR + 3 * QU:], in_=d_augLR[:, AUGR + 3 * QU:]
            )

            w1f_sb = [consts[:, 0:256], consts[:, 256:512]]
            w2f_sb = [consts[:, 512:640], consts[:, 640:768]]
            w3f_sb = consts[:, 768:832]
            chunk1 = consts[:, CH1:CH1 + BL * TQ]
            chunk0 = consts[:, CH0:CH0 + BL * TQ]

            s_tiles = {}
            w_tiles = {}
            wps = {}

            def mm1_tile(b, t0c, ncb):
                """z^T chunks t0c..t0c+ncb of batch b -> fresh zp tile."""
                zp = ps_z.tile([128, 1536], F32, tag="zp")
                for i in range(ncb):
                    c = t0c + i
                    nc.tensor.matmul(
                        zp[0:CH, SLOT[i]:SLOT[i] + T],
                        augLR[:, AUGR + b * QU + CH * c:AUGR + b * QU + CH * (c + 1)],
                        augLR[:, b * T:(b + 1) * T],
                        start=True,
                        stop=True,
                    )
                return zp

            def silu_tile(b, t0c, ncb, zp):
                """One Silu draining ncb z-chunks (ncb in {3, 6})."""
                if b not in s_tiles:
                    s_tiles[b] = work.tile(
                        [128, NCH * T], BF16, tag=f"s{b % 2}", name=f"s{b}"
                    )
                s_b = s_tiles[b]
                zin = zp[0:CH, :].rearrange("p (a r) -> p a r", r=512)[:, :, 0:400]
                zin = zin.rearrange("p a (two c) -> p two a c", two=2)
                out = s_b[0:CH, T * t0c:T * (t0c + ncb)]
                if ncb == 6:
                    nc.scalar.activation(
                        out.rearrange("p (two a c) -> p two a c", two=2, c=T),
                        zin,
                        mybir.ActivationFunctionType.Silu,
                        scale=c_dice,
                    )
                else:
                    assert ncb == 3
                    nc.scalar.activation(
                        out.rearrange("p (a c) -> p a c", c=T),
                        zin[:, 0],
                        mybir.ActivationFunctionType.Silu,
                        scale=c_dice,
                    )

            def wsel_tile(b, t0c, ncb):
                """Accumulate chunks' W2-contraction into wp (both t-chunks)."""
                if b not in wps:
                    wps[b] = ps_w.tile([128, 128], F32, tag="wp", name=f"wp{b}")
                wp = wps[b]
                s_b = s_tiles[b]
                # Single start=True on the very first matmul: its lazy
                # pending-zero covers the whole bank (ZERO_REGION=2KB), so
                # tch1's first start=False write lands on pending-zero bytes
                # and overwrites.  Per-region start=True would instead wipe
                # tch0's partial accumulation (bank-granular zeroing).
                for tch, (tc0, tlen) in enumerate(TCH):
                    for i in range(ncb):
                        c = t0c + i
                        nc.tensor.matmul(
                            wp[0:tlen, 64 * tch:64 * tch + TQ],
                            s_b[0:CH, T * c + tc0:T * c + tc0 + tlen],
                            sel[:, TQ * c:TQ * (c + 1)],
                            start=(tch == 0 and c == 0),
                            stop=(tch == 1 and c == NCH - 1),
                            skip_group_check=True,
                        )

            def wevict(b, split=False):
                w_tiles[b] = work.tile(
                    [128, 128], BF16, tag=f"w{b % 2}", name=f"w{b}"
                )
                wp = wps[b]
                nc.vector.tensor_copy(w_tiles[b][:, 0:TQ], wp[:, 0:TQ])
                if split:  # tail: second copy on the idle ACT engine
                    nc.scalar.copy(
                        w_tiles[b][0:72, 64:64 + TQ], wp[0:72, 64:64 + TQ]
                    )
                else:
                    nc.vector.tensor_copy(
                        w_tiles[b][0:72, 64:64 + TQ], wp[0:72, 64:64 + TQ]
                    )

            def g_pair(pb):
                """interest for pair (2pb, 2pb+1), pair-packed [128, 50]."""
                gp = ps_g.tile([128, 512], F32, tag="gx")
                for half in range(2):
                    b = 2 * pb + half
                    for tch, (tc0, tlen) in enumerate(TCH):
                        nc.tensor.matmul(
                            gp[64 * half:64 * half + 64, 0:TQ],
                            ubt[0:tlen, tch * BL * E + b * E:tch * BL * E + (b + 1) * E],
                            w_tiles[b][0:tlen, 64 * tch:64 * tch + TQ],
                            start=(tch == 0),
                            stop=(tch == 1),
                        )
                pair = (2 * pb, 2 * pb + 1)
                nc.vector.tensor_copy(
                    chunk0[0:64, pair[0] * TQ:(pair[0] + 1) * TQ], gp[0:64, 0:TQ]
                )
                nc.vector.tensor_copy(
                    chunk0[0:64, pair[1] * TQ:(pair[1] + 1) * TQ], gp[64:128, 0:TQ]
                )

            v_sb = {}

            def v_precompute():
                """V = ub @ w1f[0:64] for pair-1 batches (2, 3): folds the
                interest matmul (G) into the trunk's first layer so the tail
                chain skips G + g_fin entirely.  Runs mid-stream (off the
                critical path); stationary is augL rows 0:64."""
                for tch, (tc0, tlen) in enumerate(TCH):
                    v_sb[tch] = singles.tile([128, 512], BF16, name=f"v{tch}")
                    for b in (2, 3):
                        vp = ps_g.tile([128, 512], F32, tag="gx")
                        nc.tensor.matmul(
                            vp[0:tlen, 0:256],
                            augLR[0:64, b * T + tc0:b * T + tc0 + tlen],
                            consts[0:64, 0:256],
                            start=True,
                            stop=True,
                        )
                        nc.vector.tensor_copy(
                            v_sb[tch][0:tlen, (b - 2) * 256:(b - 1) * 256],
                            vp[0:tlen, 0:256],
                        )

            def trunk_tail(pb):
                """Trunk for the last pair with G folded into layer 1:
                x1 = V^T @ w + W1f[64:]^T @ [up; cx].  The up/cx matmuls have
                no late deps and run during wevict; the V matmuls follow the
                w eviction directly."""
                n0c = 2 * pb * TQ
                cols = slice(n0c, n0c + 2 * TQ)
                x1 = []
                for mch in range(2):
                    if mch == 1:
                        xp = ps_w.tile([128, 128], F32, tag="wp")
                    else:
                        xp = ps_g.tile([128, 512], F32, tag="gx")
                    nc.tensor.matmul(
                        xp[:, 0:2 * TQ],
                        w1f_sb[0][64:128, mch * 128:(mch + 1) * 128],
                        chunk0[64:128, cols], start=True, stop=False,
                    )
                    nc.tensor.matmul(
                        xp[:, 0:2 * TQ],
                        w1f_sb[1][:, mch * 128:(mch + 1) * 128],
                        chunk1[:, cols], start=False, stop=False,
                    )
                    for half in range(2):
                        b = 2 * pb + half
                        for tch, (tc0, tlen) in enumerate(TCH):
                            nc.tensor.matmul(
                                xp[:, half * TQ:(half + 1) * TQ],
                                v_sb[tch][0:tlen,
                                          half * 256 + mch * 128:
                                          half * 256 + (mch + 1) * 128],
                                w_tiles[b][0:tlen, 64 * tch:64 * tch + TQ],
                                start=False,
                                stop=(half == 1 and tch == 1),
                                skip_group_check=True,
                            )
                    x1_t = work.tile([128, 2 * TQ], BF16, tag=f"x1_{mch}")
                    relu_evict(x1_t, xp[:, 0:2 * TQ], mch == 1)
                    x1.append(x1_t)

                xp2 = ps_g.tile([128, 512], F32, tag="gx")
                nc.tensor.matmul(xp2[:, 0:2 * TQ], w2f_sb[0], x1[0],
                                 start=True, stop=False)
                nc.tensor.matmul(xp2[:, 0:2 * TQ], w2f_sb[1], x1[1],
                                 start=False, stop=True)
                x2_t = work.tile([128, 2 * TQ], BF16, tag="x2")
                relu_evict(x2_t, xp2[:, 0:2 * TQ], False)

                xp3 = ps_w.tile([64, 128], F32, tag="wp")
                nc.tensor.matmul(xp3[:, 0:2 * TQ], w3f_sb, x2_t,
                                 start=True, stop=True)
                out_t = work.tile([64, 2 * TQ], F32, tag="outT")
                relu_evict(out_t, xp3[:, 0:2 * TQ], True)
                nc.sync.dma_start(out=d_out[:, cols], in_=out_t)

            def relu_evict(out_t, xp_ap, on_act):
                if on_act:
                    nc.scalar.activation(
                        out_t, xp_ap, mybir.ActivationFunctionType.Relu
                    )
                else:
                    nc.vector.tensor_scalar_max(out_t, xp_ap, 0.0)

            def trunk_pair(pb, tail=False):
                """tail=True (last pair): second psum slot (freed wp bank) +
                ACT for half the evictions, parallelizing the stage chain."""
                n0c = 2 * pb * TQ
                cols = slice(n0c, n0c + 2 * TQ)
                x1 = []
                for mch in range(2):
                    if tail and mch == 1:
                        xp = ps_w.tile([128, 128], F32, tag="wp")
                    else:
                        xp = ps_g.tile([128, 512], F32, tag="gx")
                    nc.tensor.matmul(
                        xp[:, 0:2 * TQ],
                        w1f_sb[0][:, mch * 128:(mch + 1) * 128],
                        chunk0[:, cols], start=True, stop=False,
                    )
                    nc.tensor.matmul(
                        xp[:, 0:2 * TQ],
                        w1f_sb[1][:, mch * 128:(mch + 1) * 128],
                        chunk1[:, cols], start=False, stop=True,
                    )
                    x1_t = work.tile([128, 2 * TQ], BF16, tag=f"x1_{mch}")
                    relu_evict(x1_t, xp[:, 0:2 * TQ], tail and mch == 1)
                    x1.append(x1_t)

                xp2 = ps_g.tile([128, 512], F32, tag="gx")
                nc.tensor.matmul(xp2[:, 0:2 * TQ], w2f_sb[0], x1[0],
                                 start=True, stop=False)
                nc.tensor.matmul(xp2[:, 0:2 * TQ], w2f_sb[1], x1[1],
                                 start=False, stop=True)
                x2_t = work.tile([128, 2 * TQ], BF16, tag="x2")
                relu_evict(x2_t, xp2[:, 0:2 * TQ], False)

                if tail:
                    xp3 = ps_w.tile([64, 128], F32, tag="wp")
                else:
                    xp3 = ps_g.tile([64, 512], F32, tag="gx")
                nc.tensor.matmul(xp3[:, 0:2 * TQ], w3f_sb, x2_t,
                                 start=True, stop=True)
                out_t = work.tile([64, 2 * TQ], F32, tag="outT")
                relu_evict(out_t, xp3[:, 0:2 * TQ], tail)
                nc.sync.dma_start(out=d_out[:, cols], in_=out_t)

            def tile_step(b, t0c, ncb):
                zp = mm1_tile(b, t0c, ncb)
                silu_tile(b, t0c, ncb, zp)

            # Software-pipelined schedule.  Cross-batch: the next batch's
            # first mm1+silu tile is emitted BEFORE the previous batch's
            # trailing wsel matmuls so ACT never starves at batch borders.
            tile_step(0, 0, 3)
            tile_step(0, 3, 3)
            tile_step(0, 6, 3)
            wsel_tile(0, 0, 3)
            tile_step(0, 9, 3)
            wsel_tile(0, 3, 3)
            tile_step(0, 12, 3)
            wsel_tile(0, 6, 3)
            tile_step(1, 0, 6)
            wsel_tile(0, 9, 3)
            wsel_tile(0, 12, 3)
            wevict(0)
            tile_step(1, 6, 6)
            wsel_tile(1, 0, 6)
            tile_step(1, 12, 3)
            wsel_tile(1, 6, 6)
            tile_step(2, 0, 6)
            wsel_tile(1, 12, 3)
            wevict(1)
            g_pair(0)
            tile_step(2, 6, 6)
            wsel_tile(2, 0, 6)
            trunk_pair(0)
            v_precompute()
            tile_step(2, 12, 3)
            wsel_tile(2, 6, 6)
            tile_step(3, 0, 6)
            wsel_tile(2, 12, 3)
            wevict(2)
            tile_step(3, 6, 6)
            wsel_tile(3, 0, 6)
            tile_step(3, 12, 3)
            wsel_tile(3, 6, 6)
            wsel_tile(3, 12, 3)
            wevict(3, split=True)
            trunk_tail(1)

    nc.compile()
    return nc


def _prepare_maps(inputs):
    f = lambda k: np.ascontiguousarray(np.asarray(inputs[k], dtype=np.float32))
    W1, W2 = f("W1"), f("W2")
    b1 = f("b1")
    Wm1, Wm2, Wm3 = f("Wm1"), f("Wm2"), f("Wm3")

    A = W1[0:64] + W1[128:192]     # q rows + (q-k) rows
    Bm = W1[64:128] - W1[128:192]  # k rows - (q-k) rows
    D = W1[192:256]                # (q*k) rows
    c = 1.0 / np.sqrt(1.0 + EPS)   # dice rsqrt(var+eps) with var=1
    cb = 1.0 / np.sqrt(1.0 + EPS)  # BN identity scale

    w1f = cb * Wm1
    w2f = cb * Wm2
    w3f = cb * Wm3
    cB = np.concatenate(
        [w1f[0:128], w1f[128:256], w2f[0:128], w2f[128:256], w3f], axis=1
    )  # (128, 832)

    # Sel[(qu-row within chunk), 50c + q] = W2[u]/c * delta(q == r//36)
    M = np.zeros((QU, TQ), np.float32)
    r = np.arange(QU)
    M[r, r // U] = W2[r % U, 0] / c
    selm = M.reshape(NCH, CH, TQ).transpose(1, 0, 2).reshape(CH, NCH * TQ)

    ub = f("user_behavior")        # (B, T, E)
    it = f("items")                # (B, TQ, E)
    up = f("user_profile")         # (B, P)
    cx = f("context")              # (B, C)

    in_maps = []
    for i in range(NCORES):
        s = slice(i * BL, (i + 1) * BL)
        ub_i, it_i = ub[s], it[s]

        augLR = np.empty((65, BL * T + BL * QU), np.float32)
        for b in range(BL):
            augLR[0:64, b * T:(b + 1) * T] = ub_i[b].T
            augLR[64, b * T:(b + 1) * T] = 1.0

        itt = it_i.transpose(0, 2, 1)  # (BL, E, TQ)
        mprime = (
            itt[:, :, :, None] * D[None, :, None, :]
            + A[None, :, None, :]
        ).reshape(BL, E, QU)
        termq = (
            np.einsum("bqe,eu->bqu", it_i, Bm) + b1[None, None, :]
        ).reshape(BL, QU)
        for b in range(BL):
            cols = slice(AUGR + b * QU, AUGR + (b + 1) * QU)
            augLR[0:64, cols] = mprime[b]
            augLR[64, cols] = termq[b]

        ubt = np.zeros((128, 2 * BL * E), np.float32)
        for b in range(BL):
            ubt[0:128, b * E:(b + 1) * E] = ub_i[b, 0:128]
            ubt[0:72, BL * E + b * E:BL * E + (b + 1) * E] = ub_i[b, 128:200]

        consts = np.zeros((128, NCONST), np.float32)
        consts[:, 0:832] = cB
        for b in range(BL):
            cols = slice(CH1 + b * TQ, CH1 + (b + 1) * TQ)
            consts[0:64, cols] = up[s][b, 64:128, None]
            consts[64:128, cols] = cx[s][b, :, None]
            cols = slice(CH0 + b * TQ, CH0 + (b + 1) * TQ)
            consts[64:128, cols] = up[s][b, 0:64, None]

        in_maps.append({
            "augLR": np.ascontiguousarray(augLR.astype(BF16NP)),
            "sel": np.ascontiguousarray(selm.astype(BF16NP)),
            "ubt": np.ascontiguousarray(ubt.astype(BF16NP)),
            "consts": np.ascontiguousarray(consts.astype(BF16NP)),
        })
    return in_maps


def run(inputs, trace=False):
    if "nc" not in _CACHE:
        _CACHE["nc"] = _build_program()
    nc = _CACHE["nc"]
    in_maps = _prepare_maps(inputs)
    res = run_bass_kernel_spmd(nc, in_maps, list(range(NCORES)), trace=trace)
    out = np.empty((B, TQ, 64), dtype=np.float32)
    for i in range(NCORES):
        out[i * BL:(i + 1) * BL] = (
            res.results[i]["out"].T.reshape(BL, TQ, 64)
        )
    return out, res


def kernel(**inputs):
    out, _ = run(inputs, trace=False)
    return out
